# revision 1
# baseline (speedup 1.0000x reference)
"""Multi-head attention (B=8, N=1024, C=1024, H=16) on 8 TRN2 NeuronCores.

Data-parallel over batch: core b computes batch element b end-to-end; no
collectives. All matmuls run in bf16 with fp32 PSUM accumulation, and ALL of
them use the full 128x128 array mode (no tile_position packing) so the PE
never pays a tiling-mode-switch drain regardless of scheduler interleaving.

Per-head-pair trickery (pair p = heads 2p, 2p+1 share a 128-partition tile;
head A on partitions 0:64, head B on 64:128):

  scores  sT_h[j,i]: lhsT = kT_pair [d(128), j(128)] (both heads), rhs = qTz_h
          where qTz_A has q_A rows at 0:64 and ZEROS at 64:128 (mirrored for
          B) -> the zero rows annihilate the other head's k columns, so a
          full-mode K=128 matmul yields exactly one head's scores.
  exp     batched over 2 PSUM banks per ACTIVATE (amortizes the 352-cycle
          fixed cost), writes bf16 pT straight to SBUF in AV-ready layout.
  AV+Z    lhsT_A = [v_A | ones] [j, 128]: out rows 0:64 = attn-weighted v,
          rows 64:128 = Z (softmax denominator) REPLICATED over 64
          partitions -- the otherwise-idle half of the array computes the
          denominator and its cross-partition broadcast for free.
          lhsT_B = [ones | v_B] (ones block shared in a 192-wide
          [v_A | ones | v_B] layout).
  norm    1/Z = exp(-ln(Z)) on ScalarE (ln+exp live in ONE activation table
          set, and the ln hops the partition window: read the Z half, write
          the out half), then one tensor_mul per head fused with the bf16
          downcast into aT.
  proj    y[n,o]: lhsT = aT tile, rhs = proj_wT; bias-add fused with the
          PSUM drain. aT is split by i-half so proj of the first n-half
          overlaps attention of the second.

Tiles are deliberately split (x/w in 2-c-tile chunks, kT/qTz per pair, von
per group, aT per i-half) because Tile tracks dependencies at tile
granularity -- single big tiles serialize whole phases.
"""

import numpy as np
import ml_dtypes

import concourse.bass as bass
import concourse.tile as tile
import concourse.tile_utils as tile_utils
from concourse import bacc, mybir, bass_utils

tile_utils.max_sbuf_usage = 208 * 1024  # stale 192KiB cap; cayman has 208 usable

# Force Exp and Ln onto the one table set that holds both, otherwise the
# act-table chooser alternates exp_and_others <-> natural_log (63 reloads,
# ~1.3us each, serializing the softmax pipeline).
import concourse.hw_specs as _hw_specs


def _gat_one_set(arch, _orig=_hw_specs.get_activation_tables):
    tables = _orig(arch)
    for name, funcs in tables.items():
        if name != "natural_log_exp_and_others":
            funcs.discard(mybir.ActivationFunctionType.Exp)
            funcs.discard(mybir.ActivationFunctionType.Ln)
    return tables


bacc.get_activation_tables = _gat_one_set

N = 1024   # sequence length
C = 1024   # model dim
H = 16     # heads
D = 64     # head dim
CT = 8     # 128-row tiles of c (contraction dim)
NT = 8     # 128-row tiles of n
NB = 2     # 512-wide blocks of n
PAIRS = 8

BF16 = mybir.dt.bfloat16
F32 = mybir.dt.float32

_nc_cache = None


def build_nc():
    global _nc_cache
    if _nc_cache is not None:
        return _nc_cache

    nc = bacc.Bacc("TRN2", target_bir_lowering=False, debug=False, num_devices=8)

    x_d = nc.dram_tensor("x", [C, N], BF16, kind="ExternalInput").ap()
    qkv_w_d = nc.dram_tensor("qkv_w", [C, 3 * C], BF16, kind="ExternalInput").ap()
    proj_w_d = nc.dram_tensor("proj_w", [C, C], BF16, kind="ExternalInput").ap()
    proj_b_d = nc.dram_tensor("proj_b", [C], F32, kind="ExternalInput").ap()
    out_d = nc.dram_tensor("out", [N, C], F32, kind="ExternalOutput").ap()

    Exp = mybir.ActivationFunctionType.Exp
    Ln = mybir.ActivationFunctionType.Ln

    with tile.TileContext(nc) as tc:
        with tc.tile_pool(name="big", bufs=1) as big, \
             tc.tile_pool(name="wk", bufs=2) as wk, \
             tc.tile_pool(name="ps", bufs=2, space="PSUM") as ps:

            xT_s = [big.tile([128, 2, N], BF16, name=f"xT{i}", tag=f"x{i}")
                    for i in range(4)]
            # split by (c-chunk, q/k/v part): the first qk group only needs
            # the q+k thirds, so it completes ~3x sooner after launch
            qkv_wT_s = [[big.tile([128, 2, C], BF16, name=f"qw{i}_{s}",
                                  tag=f"qw{i}_{s}") for s in range(3)]
                        for i in range(4)]
            proj_wT_s = big.tile([128, CT, C], BF16)
            # per-pair zero-padded q (A rows 0:64 / B rows 64:128) + natural k
            qTz_s = [big.tile([128, 2, N], BF16, name=f"qz{p}", tag=f"qz{p}")
                     for p in range(PAIRS)]
            kT_s = [big.tile([128, N], BF16, name=f"kT{p}", tag=f"kT{p}")
                    for p in range(PAIRS)]
            # [v_A | ones | v_B] per (n-tile, pair), split by pair-group
            von_s = [big.tile([128, NT, 4, 3, 64], BF16, name=f"von{g}", tag=f"von{g}")
                     for g in range(2)]
            # attention output, per (pair, i-half) so proj ct-matmuls unblock
            # as soon as their pair finishes
            aT_s = [[big.tile([128, 512], BF16, name=f"aT{p}_{ib}", tag=f"aT{p}_{ib}")
                     for ib in range(NB)] for p in range(PAIRS)]
            bias_s = big.tile([128, C], F32)

            def xT(ct):
                return xT_s[ct // 2][:, ct % 2, :]

            def qw(ct, part):
                return qkv_wT_s[ct // 2][part][:, ct % 2, :]

            # one-time constant fills (gpsimd: keeps DVE/ACT free)
            for p in range(PAIRS):
                nc.gpsimd.memset(qTz_s[p][64:128, 0, :], 0.0)
                nc.gpsimd.memset(qTz_s[p][0:64, 1, :], 0.0)
            for g in range(2):
                nc.gpsimd.memset(von_s[g][:, :, :, 1, :], 1.0)

            for i in range(4):
                for s in range(2):  # q and k thirds first
                    for h in range(2):
                        nc.sync.dma_start(
                            out=qkv_wT_s[i][s][:, h, :],
                            in_=qkv_w_d[i * 256 + h * 128:i * 256 + (h + 1) * 128,
                                        s * C:(s + 1) * C])
                nc.sync.dma_start(
                    out=xT_s[i][:, 0, :], in_=x_d[i * 256:i * 256 + 128, :])
                nc.sync.dma_start(
                    out=xT_s[i][:, 1, :], in_=x_d[i * 256 + 128:(i + 1) * 256, :])
            for i in range(4):
                for h in range(2):  # v third afterwards
                    nc.sync.dma_start(
                        out=qkv_wT_s[i][2][:, h, :],
                        in_=qkv_w_d[i * 256 + h * 128:i * 256 + (h + 1) * 128,
                                    2 * C:3 * C])
            bias_bcast = bass.AP(
                tensor=proj_b_d.tensor,
                offset=proj_b_d.offset,
                ap=[[0, 128], proj_b_d.ap[0]],
            )
            nc.gpsimd.dma_start(out=bias_s, in_=bias_bcast)

            def qkv_qk(p):
                for which, ot in ((0, p), (1, 8 + p)):  # 0 = q-tile, 1 = k-tile
                    for nb in range(NB):
                        nbs = slice(nb * 512, (nb + 1) * 512)
                        acc = ps.tile([128, 512], F32, tag="qp", name=f"qk{ot}_{nb}")
                        for ct in range(CT):
                            nc.tensor.matmul(
                                acc,
                                qw(ct, which)[:, (ot % 8) * 128:(ot % 8 + 1) * 128],
                                xT(ct)[:, nbs],
                                start=(ct == 0), stop=(ct == CT - 1))
                        if which == 0:
                            nc.vector.tensor_copy(
                                out=qTz_s[p][0:64, 0, nbs], in_=acc[0:64, :])
                            nc.vector.tensor_copy(
                                out=qTz_s[p][64:128, 1, nbs], in_=acc[64:128, :])
                        else:
                            nc.vector.tensor_copy(out=kT_s[p][:, nbs], in_=acc)

            def qkv_v(g):
                # v natural layout [n, o'], o'-block g covers pairs 4g..4g+3
                for nt in range(NT):
                    acc = ps.tile([128, 512], F32, tag="qp", name=f"v{nt}_{g}")
                    for ct in range(CT):
                        nc.tensor.matmul(
                            acc,
                            xT(ct)[:, nt * 128:(nt + 1) * 128],
                            qw(ct, 2)[:, g * 512:(g + 1) * 512],
                            start=(ct == 0), stop=(ct == CT - 1))
                    # batched strided copies: all 4 pairs' A-halves, then B-halves
                    accv = acc.rearrange("p (q w e) -> p q w e", q=4, w=2)
                    nc.vector.tensor_copy(
                        out=von_s[g][:, nt, :, 0, :], in_=accv[:, :, 0, :])
                    nc.vector.tensor_copy(
                        out=von_s[g][:, nt, :, 2, :], in_=accv[:, :, 1, :])

            def attention(p, ib):
                g, q4 = p // 4, p % 4
                ibs = slice(ib * 512, (ib + 1) * 512)
                pT = [wk.tile([128, 8, 512], BF16, tag="pT",
                              name=f"pT{p}_{ib}_{h}") for h in range(2)]
                for h in range(2):
                    for jb in range(4):  # 2 j-tiles per psum batch
                        s2 = ps.tile([128, 2, 512], F32, tag="s",
                                     name=f"s{p}_{ib}_{h}_{jb}")
                        for u in range(2):
                            jt = 2 * jb + u
                            nc.tensor.matmul(
                                s2[:, u, :],
                                kT_s[p][:, jt * 128:(jt + 1) * 128],
                                qTz_s[p][:, h, ibs],
                                start=True, stop=True)
                        nc.scalar.activation(
                            out=pT[h][:, 2 * jb:2 * jb + 2, :], in_=s2,
                            func=Exp, scale=0.125)
                psA = ps.tile([128, 512], F32, tag="o", name=f"psA{p}_{ib}")
                psB = ps.tile([128, 512], F32, tag="o", name=f"psB{p}_{ib}")
                for jt in range(8):
                    nc.tensor.matmul(
                        psA, von_s[g][:, jt, q4, 0:2, :].rearrange("p a b -> p (a b)"),
                        pT[0][:, jt, :],
                        start=(jt == 0), stop=(jt == 7), skip_group_check=True)
                for jt in range(8):
                    nc.tensor.matmul(
                        psB, von_s[g][:, jt, q4, 1:3, :].rearrange("p a b -> p (a b)"),
                        pT[1][:, jt, :],
                        start=(jt == 0), stop=(jt == 7), skip_group_check=True)
                # psA rows 64:128 = Z_A replicated; psB rows 0:64 = Z_B.
                # 1/Z = exp(-ln Z) on ScalarE; the ln hops partition windows.
                lnt = wk.tile([128, 512], F32, tag="rz", bufs=4, name=f"ln{p}_{ib}")
                rz = wk.tile([128, 512], F32, tag="rz", bufs=4, name=f"rz{p}_{ib}")
                nc.scalar.activation(out=lnt[0:64, :], in_=psA[64:128, :], func=Ln)
                nc.scalar.activation(out=lnt[64:128, :], in_=psB[0:64, :], func=Ln)
                nc.scalar.activation(out=rz, in_=lnt, func=Exp, scale=-1.0)
                nc.vector.tensor_mul(
                    out=aT_s[p][ib][0:64, :], in0=psA[0:64, :], in1=rz[0:64, :])
                nc.vector.tensor_mul(
                    out=aT_s[p][ib][64:128, :], in0=psB[64:128, :],
                    in1=rz[64:128, :])

            def proj(nt, ib):
                y = wk.tile([128, C], F32, tag="y", bufs=1, name=f"y{nt}")
                for ob in range(NB):
                    obs = slice(ob * 512, (ob + 1) * 512)
                    acc = ps.tile([128, 512], F32, tag="qp", name=f"pr{nt}_{ob}")
                    for ct in range(CT):
                        nc.tensor.matmul(
                            acc,
                            aT_s[ct][ib][:, nt % 4 * 128:(nt % 4 + 1) * 128],
                            proj_wT_s[:, ct, obs],
                            start=(ct == 0), stop=(ct == CT - 1))
                    nc.vector.tensor_add(out=y[:, obs], in0=acc, in1=bias_s[:, obs])
                nc.sync.dma_start(out=out_d[nt * 128:(nt + 1) * 128, :], in_=y)

            for p in range(4):
                qkv_qk(p)
            qkv_v(0)
            for p in range(4):
                attention(p, 0)
            for p in range(4, 8):
                qkv_qk(p)
            qkv_v(1)
            for ct in range(CT):
                nc.sync.dma_start(
                    out=proj_wT_s[:, ct, :], in_=proj_w_d[ct * 128:(ct + 1) * 128, :])
            for p in range(4, 8):
                attention(p, 0)
            for p in range(4):
                attention(p, 1)
            for nt in range(4):
                proj(nt, 0)
            for p in range(4, 8):
                attention(p, 1)
            for nt in range(4, 8):
                proj(nt, 1)

    nc.finalize()
    _nc_cache = nc
    return nc


def kernel(x, qkv_w, proj_w, proj_b, trace=False):
    nc = build_nc()
    bf = ml_dtypes.bfloat16
    x = np.asarray(x, dtype=np.float32)
    qkv_wT = np.ascontiguousarray(np.asarray(qkv_w, dtype=np.float32).T).astype(bf)
    proj_wT = np.ascontiguousarray(np.asarray(proj_w, dtype=np.float32).T).astype(bf)
    proj_b = np.ascontiguousarray(np.asarray(proj_b, dtype=np.float32))

    in_maps = []
    for b in range(8):
        in_maps.append({
            "x": np.ascontiguousarray(x[b].T).astype(bf),
            "qkv_w": qkv_wT,
            "proj_w": proj_wT,
            "proj_b": proj_b,
        })

    res = bass_utils.run_bass_kernel_spmd(
        nc, in_maps, core_ids=list(range(8)), trace=trace)
    out = np.stack([
        np.asarray(res.results[b]["out"], dtype=np.float32) for b in range(8)])
    if trace:
        return out, res
    return out

